# revision 11
# baseline (speedup 1.0000x reference)
"""Single-head causal attention (B=8, T=2048, D=1024, H=128) on 8 TRN2
NeuronCores — data-parallel over batch (one batch element per core).

Per-core dataflow (bf16 matmul compute, f32 accumulation):
  0. All DMAs issue first, all x loads + output stores on the sync HWDGE
     ring (a trigger blocks its issuing sequencer ~2.7us once the ring
     backs up; the sync sequencer has no compute to starve), weights on
     the scalar ring (3 cheap early triggers). wq leads the sync ring so
     chunk-0 projections aren't weight-gated; the first tiles are split
     into column-halves for earlier transpose start. Weights arrive
     host-reshaped to [p, dt, h] so their DMA is contiguous.
  1. PE warmup matmuls flip the HAM clock-gate to 2.4 GHz during the
     x-arrival window, with a few sprinkled between the early transposes
     to keep it warm across arrival gaps.
  2. Per q-chunk c (4 t-tiles): transpose the 4 tiles on TensorE (bf16
     truncation via bitcast) into xT [d-part, d-tile, t]; project
     qT/kT/vT with N=512 matmuls (weights stationary); PE-transpose vT
     tiles into v_aug [t-tile, 129] (v natural + ones column that makes
     PV also produce the softmax denominator).
  3. Attention for chunk c is emitted immediately after its projections,
     so exp on ScalarE overlaps the next chunk's transposes/projections
     on TensorE. Scores TRANSPOSED per k-tile: ST[k 128, q 512] =
     kT_tile^T @ qT_chunk, one PSUM bank each, with a one-tile lookahead
     so the PE stays fed while ScalarE exps. Causality: lower-left tiles
     skipped, diagonal tiles exp only the valid range and zero the
     128x128 triangle via GpSimd affine_select on PT.
  4. O[q 128, 129] += PT_slice^T @ v_aug_tile accumulated over k tiles in
     PSUM (one bank per q-tile — start=True clears has_written bank-wide,
     so accumulators must not share banks); col 128 is the softmax
     denominator. Each q-tile's divide + DMA out fire at its chain stop.

  PSUM is exactly 8 persistent bank-tiles reused via WAR deps (2 shared
  transpose/projection scratch, 2 score, 4 o); SBUF work tiles are small
  fixed rings. Keeping the total pool.tile() count low matters: the
  kernel-tail drain costs ~0.1us per allocated tile.
"""

import numpy as np

import concourse.bass as bass
import concourse.bacc as bacc
import concourse.mybir as mybir
import concourse.tile as tile
from concourse import bass_utils
from concourse.masks import make_identity

B, T, D, H = 8, 2048, 1024, 128
P = 128
DT = D // P  # 8 d tiles
TT = T // P  # 16 t tiles
CH = 512  # q chunk width
QC = T // CH  # 4 q chunks
N_CORES = 8
SCALE = float(1.0 / np.sqrt(H))
N_WARMUP = 26
N_SPLIT = 3  # leading x tiles DMA'd as column-halves

F32 = mybir.dt.float32
BF16 = mybir.dt.bfloat16


def build_nc():
    nc = bacc.Bacc("TRN2", target_bir_lowering=False, debug=False)
    x = nc.dram_tensor("x", [T, D], F32, kind="ExternalInput").ap()
    wq_d = nc.dram_tensor("wq", [P, DT, H], F32, kind="ExternalInput").ap()
    wk_d = nc.dram_tensor("wk", [P, DT, H], F32, kind="ExternalInput").ap()
    wv_d = nc.dram_tensor("wv", [P, DT, H], F32, kind="ExternalInput").ap()
    out = nc.dram_tensor("out", [T, H], F32, kind="ExternalOutput").ap()

    with tile.TileContext(nc) as tc:
        _build_body(nc, tc, x, wq_d, wk_d, wv_d, out)
    nc.compile()
    return nc


def _build_body(nc, tc, x, wq_d, wk_d, wv_d, out):
    with (
        tc.tile_pool(name="persist", bufs=1) as persist,
        tc.tile_pool(name="ps", bufs=1, space="PSUM") as ps,
    ):
        # ---- all DMAs first ----
        x_nat = [
            persist.tile([P, D], F32, tag=f"x{tt}", name=f"x_nat{tt}")
            for tt in range(TT)
        ]
        wf = {
            nm: persist.tile([P, DT, H], F32, tag=f"{nm}f", name=f"{nm}_f32")
            for nm in ("wq", "wk", "wv")
        }
        nc.sync.dma_start(wf["wq"][:], wq_d)
        for tt in range(TT):
            if tt < N_SPLIT:
                hd = D // 2
                nc.sync.dma_start(x_nat[tt][:, 0:hd], x[tt * P : (tt + 1) * P, 0:hd])
                nc.sync.dma_start(x_nat[tt][:, hd:D], x[tt * P : (tt + 1) * P, hd:D])
            else:
                nc.sync.dma_start(x_nat[tt][:], x[tt * P : (tt + 1) * P, :])
        nc.scalar.dma_start(wf["wk"][:], wk_d)
        nc.scalar.dma_start(wf["wv"][:], wv_d)

        # ---- constants ----
        ident_b = persist.tile([P, P], BF16, tag="ident_b", name="ident_b")
        make_identity(nc, ident_b)

        v_aug = persist.tile([P, TT, H + 1], BF16, tag="v_aug", name="v_aug")
        nc.gpsimd.memset(v_aug[:, :, H : H + 1], 1.0)  # ones col (denominator)

        warm_src = persist.tile([P, 3 * P], BF16, tag="warm_src", name="warm_src")
        nc.vector.memset(warm_src[:], 0.5)

        # ---- PSUM: exactly 8 bank-tiles, reused for the whole kernel ----
        mm2 = [ps.tile([P, CH], F32, tag=f"mm{j}", name=f"mm{j}") for j in range(2)]
        st2 = [ps.tile([P, CH], F32, tag=f"st{j}", name=f"st{j}") for j in range(2)]
        o4 = [ps.tile([P, H + 1], F32, tag=f"o{s}", name=f"o{s}") for s in range(4)]
        mm_i = [0]

        def next_mm():
            t = mm2[mm_i[0] % 2]
            mm_i[0] += 1
            return t

        def warm(n, j0=0):
            # HAM-warming matmuls; write the (currently idle) score banks
            for w in range(n):
                nc.tensor.matmul(
                    st2[(j0 + w) % 2][:, 0 : 3 * P],
                    ident_b[:],
                    warm_src[:],
                    start=True,
                    stop=True,
                )

        warm(N_WARMUP)

        # weights cast to bf16 on DVE
        w_bf = []
        for nm in ("wq", "wk", "wv"):
            wb = persist.tile([P, DT, H], BF16, tag=f"{nm}b", name=f"{nm}_bf")
            nc.vector.tensor_copy(wb[:], wf[nm][:])
            w_bf.append(wb)
        wq_bf, wk_bf, wv_bf = w_bf

        # ---- persistent activations ----
        xT = persist.tile([P, DT, T], BF16, tag="xT", name="xT")
        qT = persist.tile([P, T], BF16, tag="qT", name="qT")
        kT = persist.tile([P, T], BF16, tag="kT", name="kT")
        vT = persist.tile([P, T], BF16, tag="vT", name="vT")

        # ---- SBUF work rings ----
        pt4 = [
            persist.tile([P, CH], BF16, tag=f"pt{j}", name=f"pt{j}") for j in range(4)
        ]
        o_sb8 = [
            persist.tile([P, H], F32, tag=f"osb{j}", name=f"o_sb{j}") for j in range(8)
        ]
        rcp8 = [
            persist.tile([P, 1], F32, tag=f"rcp{j}", name=f"rcp{j}") for j in range(8)
        ]

        for c in range(QC):
            # ---- transposes for this chunk's 4 t-tiles ----
            for tt in range(4 * c, 4 * c + 4):
                xv = x_nat[tt].bitcast(BF16)  # truncation view (hi 2B of f32)
                for half in range(2):
                    trt = next_mm()
                    trv = trt.bitcast(BF16)  # [P, 2*CH] bf16 view of the bank
                    for j in range(4):
                        dt = half * 4 + j
                        nc.tensor.transpose(
                            trv[:, j * P : (j + 1) * P],
                            xv[:, 2 * dt * P + 1 : 2 * (dt + 1) * P : 2],
                            ident_b,
                        )
                    dst = xT[:, half * 4 : half * 4 + 4, tt * P : (tt + 1) * P]
                    src = trv[:, 0 : 4 * P].rearrange("p (a t) -> p a t", a=4)
                    if (tt + half) % 2 == 0:
                        nc.vector.tensor_copy(dst, src)
                    else:
                        nc.scalar.copy(dst, src)
                if c == 0 and tt < 3:
                    warm(3)  # keep HAM hot across x-arrival gaps
            # ---- projections for this chunk, N=512, weights stationary ----
            t0 = c * CH
            for nm, wb, dstT in (
                ("q", wq_bf, qT),
                ("k", wk_bf, kT),
                ("v", wv_bf, vT),
            ):
                pr = next_mm()
                for dt in range(DT):
                    nc.tensor.matmul(
                        pr[:],
                        wb[:, dt, :],
                        xT[:, dt, t0 : t0 + CH],
                        start=(dt == 0),
                        stop=(dt == DT - 1),
                    )
                if nm == "v":
                    nc.scalar.copy(dstT[:, t0 : t0 + CH], pr[:])
                else:
                    nc.vector.tensor_copy(dstT[:, t0 : t0 + CH], pr[:])
            # v natural tiles: PE-transpose vT tiles into v_aug
            for tt in range(4 * c, 4 * c + 4):
                vtr = next_mm().bitcast(BF16)
                nc.tensor.transpose(
                    vtr[:, 0:P], vT[:, tt * P : (tt + 1) * P], ident_b
                )
                nc.vector.tensor_copy(v_aug[:, tt, 0:H], vtr[:, 0:P])

            # ---- attention for this chunk, one k-tile at a time ----
            n_k = 4 * c + 4  # k-tiles 0..4c+3

            def emit_s(i, c=c):
                st = st2[i % 2]
                e0 = max(i - 4 * c, 0) * P
                nc.tensor.matmul(
                    st[:, e0:CH],
                    kT[:, i * P : (i + 1) * P],
                    qT[:, c * CH + e0 : (c + 1) * CH],
                    start=True,
                    stop=True,
                )
                return st

            emit_s(0)
            for i in range(n_k):
                if i + 1 < n_k:
                    emit_s(i + 1)  # keep PE fed while ACT does exp(i)
                st = st2[i % 2]
                pt = pt4[i % 4]
                j0 = i - 4 * c  # diag offset (>=0 on the diagonal tile)
                e0 = max(j0, 0) * P
                nc.scalar.activation(
                    pt[:, e0:CH],
                    st[:, e0:CH],
                    mybir.ActivationFunctionType.Exp,
                    scale=SCALE,
                )
                if j0 >= 0:
                    # zero the causal triangle of the diagonal block
                    nc.gpsimd.affine_select(
                        out=pt[:, e0 : e0 + P],
                        in_=pt[:, e0 : e0 + P],
                        compare_op=mybir.AluOpType.is_ge,
                        fill=0.0,
                        base=0,
                        pattern=[[1, P]],
                        channel_multiplier=-1,
                    )
                for s in range(4):
                    if i <= 4 * c + s:
                        nc.tensor.matmul(
                            o4[s][:],
                            pt[:, s * P : (s + 1) * P],
                            v_aug[:, i, :],
                            start=(i == 0),
                            stop=(i == 4 * c + s),
                        )
                # q-tile whose accumulation chain just stopped: divide + out
                s = i - 4 * c
                if 0 <= s < 4:
                    qt_idx = 4 * c + s
                    recip = rcp8[qt_idx % 8]
                    nc.vector.reciprocal(recip[:], o4[s][:, H : H + 1])
                    o_sb = o_sb8[qt_idx % 8]
                    nc.vector.tensor_scalar_mul(o_sb[:], o4[s][:, 0:H], recip[:])
                    # stores on the sync ring only (see DMA note above)
                    nc.sync.dma_start(out[qt_idx * P : (qt_idx + 1) * P, :], o_sb[:])


_NC_CACHE = None


def _get_nc():
    global _NC_CACHE
    if _NC_CACHE is None:
        _NC_CACHE = build_nc()
    return _NC_CACHE


def kernel(**inputs):
    x = np.ascontiguousarray(np.asarray(inputs["x"], dtype=np.float32))

    def host_reshape(w):
        # [D, H] -> [p, dt, h] (pure layout permutation, dtype preserved)
        w = np.asarray(w, dtype=np.float32)
        return np.ascontiguousarray(w.reshape(DT, P, H).transpose(1, 0, 2))

    wq = host_reshape(inputs["Wq"])
    wk = host_reshape(inputs["Wk"])
    wv = host_reshape(inputs["Wv"])
    assert x.shape == (B, T, D)
    nc = _get_nc()
    in_maps = [
        {"x": np.ascontiguousarray(x[b]), "wq": wq, "wk": wk, "wv": wv}
        for b in range(N_CORES)
    ]
    res = bass_utils.run_bass_kernel_spmd(nc, in_maps, core_ids=list(range(N_CORES)))
    return np.stack([res.results[b]["out"] for b in range(N_CORES)], axis=0)


# revision 13
# speedup vs baseline: 1.4927x; 1.4927x over previous
"""Single-head causal attention (B=8, T=2048, D=1024, H=128) on 8 TRN2
NeuronCores — data-parallel over batch (one batch element per core).

Per-core dataflow (bf16 matmul compute, f32 accumulation):
  0. All DMAs issue first, all x loads + output stores on the sync HWDGE
     ring (a trigger blocks its issuing sequencer ~2.7us once the ring
     backs up; the sync sequencer has no compute to starve), weights on
     the scalar ring (3 cheap early triggers). wq leads the sync ring so
     chunk-0 projections aren't weight-gated; the first tiles are split
     into column-halves for earlier transpose start. Weights arrive
     host-reshaped to [p, dt, h] so their DMA is contiguous.
  1. PE warmup matmuls flip the HAM clock-gate to 2.4 GHz during the
     x-arrival window, with a few sprinkled between the early transposes
     to keep it warm across arrival gaps. All warmups write one tile:
     same-engine WAW needs no semaphores.
  2. Per q-chunk c (4 t-tiles): transpose the 4 tiles on TensorE (bf16
     truncation via bitcast) into xT [d-part, d-tile, t]; project
     qT/kT/vT with N=512 matmuls (weights stationary); PE-transpose vT
     tiles into v_aug [t-tile, 129] (v natural + ones column that makes
     PV also produce the softmax denominator).
  3. Attention for chunk c is emitted immediately after its projections,
     so exp on ScalarE overlaps the next chunk's transposes/projections
     on TensorE. Scores TRANSPOSED per k-tile: ST[k 128, q 512] =
     kT_tile^T @ qT_chunk, one PSUM bank each, pool-double-buffered with
     a one-tile lookahead so the PE stays fed while ScalarE exps.
     Causality: lower-left tiles skipped, diagonal tiles exp only the
     valid range and zero the 128x128 triangle via GpSimd affine_select.
  4. O[q 128, 129] += PT_slice^T @ v_aug_tile accumulated over k tiles in
     PSUM (one bank per q-tile — start=True clears has_written bank-wide,
     so accumulators must not share banks); col 128 is the softmax
     denominator. Each q-tile's divide + DMA out fire at its chain stop.

  NOTE: hot-path tiles come from pools (tag + bufs) on purpose — a pool
  slot hand-off aggregates all readers into one release edge, while
  direct tile reuse makes every writer wait on each prior reader and the
  per-instruction semaphore traffic inflates the whole kernel ~15%.
"""

import numpy as np

import concourse.bass as bass
import concourse.bacc as bacc
import concourse.mybir as mybir
import concourse.tile as tile
from concourse import bass_utils
from concourse.masks import make_identity

B, T, D, H = 8, 2048, 1024, 128
P = 128
DT = D // P  # 8 d tiles
TT = T // P  # 16 t tiles
CH = 512  # q chunk width
QC = T // CH  # 4 q chunks
N_CORES = 8
SCALE = float(1.0 / np.sqrt(H))
N_WARMUP = 26
N_SPLIT = 3  # leading x tiles DMA'd as column-halves

F32 = mybir.dt.float32
BF16 = mybir.dt.bfloat16


def build_nc():
    nc = bacc.Bacc("TRN2", target_bir_lowering=False, debug=False)
    x = nc.dram_tensor("x", [T, D], F32, kind="ExternalInput").ap()
    wq_d = nc.dram_tensor("wq", [P, DT, H], F32, kind="ExternalInput").ap()
    wk_d = nc.dram_tensor("wk", [P, DT, H], F32, kind="ExternalInput").ap()
    wv_d = nc.dram_tensor("wv", [P, DT, H], F32, kind="ExternalInput").ap()
    out = nc.dram_tensor("out", [T, H], F32, kind="ExternalOutput").ap()

    with tile.TileContext(nc) as tc:
        _build_body(nc, tc, x, wq_d, wk_d, wv_d, out)
    nc.compile()
    return nc


def _build_body(nc, tc, x, wq_d, wk_d, wv_d, out):
    with (
        tc.tile_pool(name="persist", bufs=1) as persist,
        tc.tile_pool(name="xpool", bufs=TT) as xpool,
        tc.tile_pool(name="work", bufs=4) as work,
        tc.tile_pool(name="ps", bufs=1, space="PSUM") as ps,
    ):
        # ---- all DMAs first ----
        x_nat = [
            xpool.tile([P, D], F32, tag="x_nat", name=f"x_nat{tt}")
            for tt in range(TT)
        ]
        wf = {}
        for nm, wd, eng in (
            ("wq", wq_d, nc.sync),
            ("wk", wk_d, nc.scalar),
            ("wv", wv_d, nc.scalar),
        ):
            wf[nm] = work.tile([P, DT, H], F32, tag="wf32", name=f"{nm}_f32")
        nc.sync.dma_start(wf["wq"][:], wq_d)
        for tt in range(TT):
            if tt < N_SPLIT:
                hd = D // 2
                nc.sync.dma_start(x_nat[tt][:, 0:hd], x[tt * P : (tt + 1) * P, 0:hd])
                nc.sync.dma_start(x_nat[tt][:, hd:D], x[tt * P : (tt + 1) * P, hd:D])
            else:
                nc.sync.dma_start(x_nat[tt][:], x[tt * P : (tt + 1) * P, :])
        nc.scalar.dma_start(wf["wk"][:], wk_d)
        nc.scalar.dma_start(wf["wv"][:], wv_d)

        # ---- constants ----
        ident_b = persist.tile([P, P], BF16, tag="ident_b", name="ident_b")
        make_identity(nc, ident_b)

        v_aug = persist.tile([P, TT, H + 1], BF16, tag="v_aug", name="v_aug")
        nc.gpsimd.memset(v_aug[:, :, H : H + 1], 1.0)  # ones col (denominator)

        warm_src = persist.tile([P, 3 * P], BF16, tag="warm_src", name="warm_src")
        nc.vector.memset(warm_src[:], 0.5)

        # single warm tile: back-to-back PE WAW needs no semaphores
        warm_ps = ps.tile([P, 3 * P], F32, tag="st", bufs=2, name="warm_ps")

        def warm(n):
            for _ in range(n):
                nc.tensor.matmul(
                    warm_ps[:], ident_b[:], warm_src[:], start=True, stop=True
                )

        warm(N_WARMUP)

        # weights cast to bf16 on DVE
        w_bf = []
        for nm in ("wq", "wk", "wv"):
            wb = persist.tile([P, DT, H], BF16, tag=f"{nm}b", name=f"{nm}_bf")
            nc.vector.tensor_copy(wb[:], wf[nm][:])
            w_bf.append(wb)
        wq_bf, wk_bf, wv_bf = w_bf

        # ---- persistent activations ----
        xT = persist.tile([P, DT, T], BF16, tag="xT", name="xT")
        qT = persist.tile([P, T], BF16, tag="qT", name="qT")
        kT = persist.tile([P, T], BF16, tag="kT", name="kT")
        vT = persist.tile([P, T], BF16, tag="vT", name="vT")

        for c in range(QC):
            # ---- transposes for this chunk's 4 t-tiles ----
            for tt in range(4 * c, 4 * c + 4):
                xv = x_nat[tt].bitcast(BF16)  # truncation view (hi 2B of f32)
                for half in range(2):
                    tr_ps = ps.tile(
                        [P, 4 * P], BF16, tag="mm", bufs=2, name=f"tr{tt}_{half}"
                    )
                    for j in range(4):
                        dt = half * 4 + j
                        nc.tensor.transpose(
                            tr_ps[:, j * P : (j + 1) * P],
                            xv[:, 2 * dt * P + 1 : 2 * (dt + 1) * P : 2],
                            ident_b,
                        )
                    dst = xT[:, half * 4 : half * 4 + 4, tt * P : (tt + 1) * P]
                    src = tr_ps.rearrange("p (a t) -> p a t", a=4)
                    if (tt + half) % 2 == 0:
                        nc.vector.tensor_copy(dst, src)
                    else:
                        nc.scalar.copy(dst, src)
                if c == 0 and tt < 3:
                    warm(3)  # keep HAM hot across x-arrival gaps
            # ---- projections for this chunk, N=512, weights stationary ----
            t0 = c * CH
            for nm, wb, dstT in (
                ("q", wq_bf, qT),
                ("k", wk_bf, kT),
                ("v", wv_bf, vT),
            ):
                pr_ps = ps.tile([P, CH], F32, tag="mm", bufs=2, name=f"{nm}T_ps{c}")
                for dt in range(DT):
                    nc.tensor.matmul(
                        pr_ps[:],
                        wb[:, dt, :],
                        xT[:, dt, t0 : t0 + CH],
                        start=(dt == 0),
                        stop=(dt == DT - 1),
                    )
                if nm == "v":
                    nc.scalar.copy(dstT[:, t0 : t0 + CH], pr_ps[:])
                else:
                    nc.vector.tensor_copy(dstT[:, t0 : t0 + CH], pr_ps[:])
            # v natural tiles: PE-transpose vT tiles into v_aug
            for tt in range(4 * c, 4 * c + 4):
                vtr = ps.tile([P, P], BF16, tag="mm", bufs=2, name=f"vtr{tt}")
                nc.tensor.transpose(vtr[:], vT[:, tt * P : (tt + 1) * P], ident_b)
                nc.vector.tensor_copy(v_aug[:, tt, 0:H], vtr[:])

            # ---- attention for this chunk, one k-tile at a time ----
            n_k = 4 * c + 4  # k-tiles 0..4c+3
            o_ps = [
                ps.tile([P, H + 1], F32, tag="o", bufs=4, name=f"o{c}_{s}")
                for s in range(4)
            ]
            st_ps = {}

            def emit_s(i, c=c, st_ps=st_ps):
                st = ps.tile([P, CH], F32, tag="st", bufs=2, name=f"st{c}_{i}")
                e0 = max(i - 4 * c, 0) * P
                nc.tensor.matmul(
                    st[:, e0:CH],
                    kT[:, i * P : (i + 1) * P],
                    qT[:, c * CH + e0 : (c + 1) * CH],
                    start=True,
                    stop=True,
                )
                st_ps[i] = st

            emit_s(0)
            for i in range(n_k):
                if i + 1 < n_k:
                    emit_s(i + 1)  # keep PE fed while ACT does exp(i)
                st = st_ps.pop(i)
                pt = work.tile([P, CH], BF16, tag="pt", name=f"pt{c}_{i}")
                j0 = i - 4 * c  # diag offset (>=0 on the diagonal tile)
                e0 = max(j0, 0) * P
                nc.scalar.activation(
                    pt[:, e0:CH],
                    st[:, e0:CH],
                    mybir.ActivationFunctionType.Exp,
                    scale=SCALE,
                )
                if j0 >= 0:
                    # zero the causal triangle of the diagonal block
                    nc.gpsimd.affine_select(
                        out=pt[:, e0 : e0 + P],
                        in_=pt[:, e0 : e0 + P],
                        compare_op=mybir.AluOpType.is_ge,
                        fill=0.0,
                        base=0,
                        pattern=[[1, P]],
                        channel_multiplier=-1,
                    )
                for s in range(4):
                    if i <= 4 * c + s:
                        nc.tensor.matmul(
                            o_ps[s][:],
                            pt[:, s * P : (s + 1) * P],
                            v_aug[:, i, :],
                            start=(i == 0),
                            stop=(i == 4 * c + s),
                        )
                # q-tile whose accumulation chain just stopped: divide + out
                s = i - 4 * c
                if 0 <= s < 4:
                    qt_idx = 4 * c + s
                    recip = work.tile(
                        [P, 1], F32, tag="recip", bufs=16, name=f"rcp{qt_idx}"
                    )
                    nc.vector.reciprocal(recip[:], o_ps[s][:, H : H + 1])
                    o_sb = work.tile(
                        [P, H], F32, tag="o_sb", bufs=16, name=f"o_sb{qt_idx}"
                    )
                    nc.vector.tensor_scalar_mul(o_sb[:], o_ps[s][:, 0:H], recip[:])
                    # stores on the sync ring only (see DMA note above)
                    nc.sync.dma_start(out[qt_idx * P : (qt_idx + 1) * P, :], o_sb[:])


_NC_CACHE = None


def _get_nc():
    global _NC_CACHE
    if _NC_CACHE is None:
        _NC_CACHE = build_nc()
    return _NC_CACHE


def kernel(**inputs):
    x = np.ascontiguousarray(np.asarray(inputs["x"], dtype=np.float32))

    def host_reshape(w):
        # [D, H] -> [p, dt, h] (pure layout permutation, dtype preserved)
        w = np.asarray(w, dtype=np.float32)
        return np.ascontiguousarray(w.reshape(DT, P, H).transpose(1, 0, 2))

    wq = host_reshape(inputs["Wq"])
    wk = host_reshape(inputs["Wk"])
    wv = host_reshape(inputs["Wv"])
    assert x.shape == (B, T, D)
    nc = _get_nc()
    in_maps = [
        {"x": np.ascontiguousarray(x[b]), "wq": wq, "wk": wk, "wv": wv}
        for b in range(N_CORES)
    ]
    res = bass_utils.run_bass_kernel_spmd(nc, in_maps, core_ids=list(range(N_CORES)))
    return np.stack([res.results[b]["out"] for b in range(N_CORES)], axis=0)
